# revision 1
# baseline (speedup 1.0000x reference)
"""TRN2 Bass kernel: nn_MultiHeadAttention (RoPE + causal) on 8 NeuronCores.

Sharding: tensor-parallel over heads (16 heads -> 2 heads/core); both batch
rows on every core.  Each core computes the QKV projection for its 2 heads,
RoPE, causal attention, and a partial output projection (its heads'
contribution to all 1024 output columns).  The host sums the 8 partials and
adds the bias.

Device dataflow (per core); "T" tensors are [feature, token] layouts:
  xT [1024, 4096] bf16    (host-pretransposed, host-cast x)
  consts blob [128, 8576] bf16 - all weights/tables in ONE DMA (keeps
        consumer instructions within the ISA sync-wait limits)
  qT/kT [128, 4096] bf16  rows = [head0 dims | head1 dims]
  RoPE: rot(q) on the PE via a block-diag rotation matrix; qT = q*cos + rot*sin
  vT [128, 4096] bf16 -> PE-transposed into V rows [128, 32, 130] with a ones
        column per head (softmax denominator falls out of P^T @ [V|1])
  S^T blocks [k=128, q<=512] = K Q^T, one C=64 matmul per head, heads packed
        into disjoint PE row groups via tile_position
  P^T = exp(S^T/8) (ScalarE, bf16); causal = block skip + triangle mask
  num^T [65, 512] f32 PSUM accumulated over k blocks; row 64 = denominator
  attn^T = num^T[0:64] * recip(den); recip via ACT ln->exp; bcast via a
        DRAM-bounce 0-stride DMA
  y[n, m] partial = attn^T-chunk.T @ w_proj-slice -> bf16 -> HBM
"""

import os
import sys

import numpy as np

os.environ.setdefault("MYCRO_LOCAL_CACHE", "1")

D_MODEL = 1024
NUM_HEADS = 16
HEAD_DIM = 64
B = 2
S = 2048
N = B * S  # 4096
NCORES = 8
ROPE_BASE = 10000.0
SM_SCALE = HEAD_DIM ** -0.5  # 0.125

# consts blob column offsets (bf16, [128, CW])
OQ, OK, OV = 0, 1024, 2048
OP = 3072          # w_proj slice [128, 1024]
OC = 4096          # cos [128, 2048]
OS = 6144          # sin [128, 2048]
OT = 8192          # triangle mask [128, 128]
OR = 8320          # rmatT [128, 128]
OI = 8448          # identity [128, 128]
CW = 8576

KERNEL_TRACE = False
LAST_RESULT = None


def _import_concourse():
    try:
        import concourse.bass  # noqa: F401
    except ImportError:
        sys.path.insert(0, "/opt/trn_rl_repo")
        import concourse.bass  # noqa: F401


_NC = None


def _split_excess_waits(nc, cap=2):
    """Walrus codegen rejects instructions with too many sync-wait commands
    (compact ISA structs have ~2 wait slots).  Split excess waits onto pure
    EventSemaphore instructions inserted just before, on the same engine —
    semantically identical (engine stalls at the same point)."""
    import concourse.mybir as mybir

    total = 0
    for fn in nc.m.functions:
        for blk in fn.blocks:
            out = []
            changed = False
            for inst in blk.instructions:
                si = inst.sync_info
                waits = list(si.on_wait) if si is not None else []
                if len(waits) > cap:
                    excess, keep = waits[:-cap], waits[-cap:]
                    for j, w in enumerate(excess):
                        out.append(
                            mybir.InstEventSemaphore(
                                name=f"{inst.name}_xw{j}",
                                engine=inst.engine,
                                ins=[],
                                outs=[],
                                sync_info=mybir.SyncInfo(on_wait=[w], on_update=[]),
                            )
                        )
                        total += 1
                    inst.sync_info = mybir.SyncInfo(
                        on_wait=keep, on_update=list(si.on_update)
                    )
                    changed = True
                out.append(inst)
            if changed:
                blk.instructions = out
    return total


def _build_bass(blk_bufs=2, pp_bufs=16, work_bufs=10, xp_bufs=2, num_bufs=2,
                rope_gpsimd=False, yb_split=True, qkv_copies_act=False,
                divide_dve=False, order=9, vr_dve=False,
                small_bufs=3, dr_bufs=4):
    """Build the (core-independent) Bass program once."""
    import concourse.bass as bass
    import concourse.mybir as mybir
    from concourse.tile import TileContext

    f32 = mybir.dt.float32
    bf16 = mybir.dt.bfloat16
    MUL = mybir.AluOpType.mult
    ADD = mybir.AluOpType.add
    EXP = mybir.ActivationFunctionType.Exp
    LN = mybir.ActivationFunctionType.Ln

    nc = bass.Bass()

    xT_h = nc.declare_dram_parameter("xT", [D_MODEL, N], bf16, isOutput=False)
    cst_h = nc.declare_dram_parameter("consts", [128, CW], bf16, isOutput=False)
    y_h = nc.declare_dram_parameter("y", [N, D_MODEL], bf16, isOutput=True)

    with TileContext(nc) as tc:
        with (
            tc.tile_pool(name="res", bufs=1) as res,
            tc.tile_pool(name="xp", bufs=xp_bufs) as xp,
            tc.tile_pool(name="work", bufs=work_bufs) as work,
            tc.tile_pool(name="pp", bufs=pp_bufs) as pp,
            tc.tile_pool(name="small", bufs=small_bufs) as small,
            tc.tile_pool(name="dr", bufs=dr_bufs, space="DRAM") as dr,
            tc.tile_pool(name="ps", bufs=4, space="PSUM") as ps,
        ):
            cst = res.tile([128, CW], bf16, name="cst")
            nc.sync.dma_start(cst[:, 0:OP], cst_h[:, 0:OP])
            nc.sync.dma_start(cst[:, OP:CW], cst_h[:, OP:CW])

            def wq_c(ch):
                return cst[:, OQ + ch * 128 : OQ + (ch + 1) * 128]

            def wk_c(ch):
                return cst[:, OK + ch * 128 : OK + (ch + 1) * 128]

            def wv_c(ch):
                return cst[:, OV + ch * 128 : OV + (ch + 1) * 128]

            wp_sb = cst[:, OP : OP + 1024]
            cos_sb = cst[:, OC : OC + 2048]
            sin_sb = cst[:, OS : OS + 2048]
            tri_sb = cst[:, OT : OT + 128]
            rm_sb = cst[:, OR : OR + 128]
            id_sb = cst[:, OI : OI + 128]

            qT = res.tile([128, N], bf16, name="qT")
            kT = res.tile([128, N], bf16, name="kT")
            vTb = res.tile([128, N], bf16, name="vTb")
            Vr = res.tile([128, 32, 130], bf16, name="Vr")
            attn = res.tile([128, N], bf16, name="attn")

            nc.vector.memset(Vr[:, :, 64], 1.0)
            nc.vector.memset(Vr[:, :, 129], 1.0)

            xT_r = xT_h[:].rearrange("(c p) n -> p c n", p=128)

            def rope_evict(psum, dst, s0, cpeng):
                # dst = q*cos + (R q)*sin ; R q via PE with resident rmatT
                qb_t = work.tile([128, 512], bf16, tag="qb")
                cpeng(qb_t, psum)
                rps = ps.tile([128, 512], f32, tag="num1", bufs=num_bufs, name="rps")
                nc.tensor.matmul(rps, lhsT=rm_sb, rhs=qb_t, start=True, stop=True)
                t1 = work.tile([128, 512], bf16, tag="t1")
                eng_t1 = nc.gpsimd if rope_gpsimd else nc.vector
                eng_t1.tensor_tensor(t1, qb_t, cos_sb[:, s0 : s0 + 512], MUL)
                t2 = work.tile([128, 512], bf16, tag="t2")
                nc.vector.tensor_tensor(t2, rps, sin_sb[:, s0 : s0 + 512], MUL)
                nc.vector.tensor_tensor(dst, t1, t2, ADD)

            # --- per batch: QKV projections + attention (interleaved) ---
            def xt_load(nb):
                nsl = slice(nb * 512, (nb + 1) * 512)
                xt = xp.tile([128, 8, 512], bf16, tag="xt")
                nc.sync.dma_start(xt[:, 0:4], xT_r[:, 0:4, nsl])
                nc.sync.dma_start(xt[:, 4:8], xT_r[:, 4:8, nsl])
                return xt

            def qkv_block(nb, xt=None):
                if qkv_copies_act == "split":
                    use_act = nb < 4
                else:
                    use_act = bool(qkv_copies_act)
                cpeng = nc.scalar.copy if use_act else nc.vector.tensor_copy
                nsl = slice(nb * 512, (nb + 1) * 512)
                if xt is None:
                    xt = xt_load(nb)
                s0 = (nb % 4) * 512

                qps = ps.tile([128, 512], f32, tag="blk", bufs=blk_bufs, name="qps")
                for ch in range(8):
                    nc.tensor.matmul(
                        qps, lhsT=wq_c(ch), rhs=xt[:, ch],
                        start=(ch == 0), stop=(ch == 7),
                    )
                rope_evict(qps, qT[:, nsl], s0, cpeng)

                kps = ps.tile([128, 512], f32, tag="blk", bufs=blk_bufs, name="kps")
                for ch in range(8):
                    nc.tensor.matmul(
                        kps, lhsT=wk_c(ch), rhs=xt[:, ch],
                        start=(ch == 0), stop=(ch == 7),
                    )
                rope_evict(kps, kT[:, nsl], s0, cpeng)

                vps = ps.tile([128, 512], f32, tag="blk", bufs=blk_bufs, name="vps")
                for ch in range(8):
                    nc.tensor.matmul(
                        vps, lhsT=wv_c(ch), rhs=xt[:, ch],
                        start=(ch == 0), stop=(ch == 7),
                    )
                cpeng(vTb[:, nsl], vps)

                for j in range(4):
                    nt = nb * 4 + j
                    vtr = ps.tile([128, 128], bf16, tag="num0", bufs=num_bufs, name="vtr")
                    nc.tensor.transpose(
                        vtr, vTb[:, nt * 128 : (nt + 1) * 128], id_sb
                    )
                    cp = nc.vector.tensor_copy if vr_dve else cpeng
                    cp(Vr[:, nt, 0:64], vtr[:, 0:64])
                    cp(Vr[:, nt, 65:129], vtr[:, 64:128])

            def attn_block(b, qb):
                    q0 = b * S + qb * 512
                    nums = [
                        ps.tile([65, 512], f32, tag=f"num{h}", bufs=num_bufs, name=f"num{h}")
                        for h in range(2)
                    ]
                    nkb = qb * 4 + 4
                    for kb in range(nkb):
                        k0 = b * S + kb * 128
                        koff = kb * 128 - qb * 512
                        js = max(0, koff)
                        diag = koff >= 0
                        s2 = ps.tile([128, 2, 512], f32, tag="blk", bufs=blk_bufs, name="s2")
                        for h in range(2):
                            nc.tensor.matmul(
                                s2[:, h, js:],
                                lhsT=kT[64 * h : 64 * h + 64, k0 : k0 + 128],
                                rhs=qT[64 * h : 64 * h + 64, q0 + js : q0 + 512],
                                start=True,
                                stop=True,
                                tile_position=(64 * h, 0),
                            )
                        pbf = pp.tile([128, 2, 512], bf16, tag="pbf")
                        nc.scalar.activation(
                            pbf[:, :, js:], s2[:, :, js:], EXP, scale=SM_SCALE
                        )
                        if diag:
                            nc.vector.tensor_tensor(
                                pbf[:, :, js : js + 128],
                                pbf[:, :, js : js + 128],
                                tri_sb[:, None, :].to_broadcast((128, 2, 128)),
                                MUL,
                            )
                        nt = b * 16 + kb
                        for h in range(2):
                            nc.tensor.matmul(
                                nums[h][:, js:],
                                lhsT=Vr[:, nt, 65 * h : 65 * h + 65],
                                rhs=pbf[:, h, js:],
                                start=(kb == 0),
                                stop=(kb == nkb - 1),
                            )
                    # softmax denominator -> reciprocal -> normalize into attn
                    for h in range(2):
                        if divide_dve is True:
                            # den row -> DRAM -> [128,4] -> DVE exact recip ->
                            # DRAM -> 0-stride broadcast.  Keeps ACT free.
                            dsb = small.tile([65, 512], f32, tag="lnt")
                            nc.vector.tensor_copy(dsb[64:65], nums[h][64:65, :])
                            dden = dr.tile([1, 512], f32, tag="dden")
                            nc.sync.dma_start(dden, dsb[64:65, :])
                            drs = small.tile([128, 4], f32, tag="drs")
                            nc.sync.dma_start(
                                drs, dden[0].rearrange("(o p) -> p o", p=128)
                            )
                            rrs = small.tile([128, 4], f32, tag="rrs")
                            nc.vector.reciprocal(rrs, drs)
                            drec = dr.tile([1, 512], f32, tag="drec")
                            nc.sync.dma_start(
                                drec[0].rearrange("(o p) -> p o", p=128), rrs
                            )
                            rb = work.tile([64, 512], f32, tag="rb")
                            nc.sync.dma_start(rb, drec[0].partition_broadcast(64))
                        elif divide_dve == "approx":
                            # DVE approx reciprocal (custom-DVE, ~2 ULP)
                            rec = small.tile([65, 512], f32, tag="rec")
                            scr = small.tile([65, 512], f32, tag="scr")
                            nc.vector.reciprocal_approx_accurate(
                                rec[64:65], nums[h][64:65, :], scr[64:65]
                            )
                            dden = dr.tile([1, 512], f32, tag="dden")
                            nc.sync.dma_start(dden, rec[64:65, :])
                            rb = work.tile([64, 512], f32, tag="rb")
                            nc.sync.dma_start(rb, dden[0].partition_broadcast(64))
                        else:
                            # single-lane ln/exp stay partition-aligned (lane 64)
                            lnt = small.tile([65, 512], f32, tag="lnt")
                            nc.scalar.activation(lnt[64:65], nums[h][64:65, :], LN)
                            rec = small.tile([65, 512], f32, tag="rec")
                            nc.scalar.activation(rec[64:65], lnt[64:65], EXP, scale=-1.0)
                            dden = dr.tile([1, 512], f32, tag="dden")
                            nc.sync.dma_start(dden, rec[64:65, :])
                            rb = work.tile([64, 512], f32, tag="rb")
                            nc.sync.dma_start(rb, dden[0].partition_broadcast(64))
                        nc.vector.tensor_tensor(
                            attn[64 * h : 64 * h + 64, q0 : q0 + 512],
                            nums[h][0:64, :],
                            rb,
                            MUL,
                        )
                    # output projection for the 4 row-tiles of this q-block
                    for i in range(4):
                        nt = q0 // 128 + i
                        for mb in range(2):
                            yps = ps.tile([128, 512], f32, tag=f"num{mb}", bufs=num_bufs, name="yps")
                            nc.tensor.matmul(
                                yps,
                                lhsT=attn[:, nt * 128 : (nt + 1) * 128],
                                rhs=wp_sb[:, mb * 512 : (mb + 1) * 512],
                                start=True,
                                stop=True,
                            )
                            yb = work.tile([128, 512], bf16, tag="yb")
                            if yb_split == "b0" and b == 0 and mb == 1:
                                nc.scalar.copy(yb, yps)
                            elif yb_split is True and mb == 1:
                                nc.scalar.copy(yb, yps)
                            else:
                                nc.vector.tensor_copy(yb, yps)
                            nc.sync.dma_start(
                                y_h[nt * 128 : (nt + 1) * 128, mb * 512 : (mb + 1) * 512],
                                yb,
                            )

            if order == 0:  # per-batch, ascending qb
                for b in range(2):
                    for nb in range(4 * b, 4 * b + 4):
                        qkv_block(nb)
                    for qb in range(4):
                        attn_block(b, qb)
            elif order == 1:  # per-batch, descending qb (long chains first)
                for b in range(2):
                    for nb in range(4 * b, 4 * b + 4):
                        qkv_block(nb)
                    for qb in (3, 2, 1, 0):
                        attn_block(b, qb)
            elif order == 2:  # all QKV, then batch-interleaved attn desc
                for nb in range(8):
                    qkv_block(nb)
                for qb in (3, 2, 1, 0):
                    attn_block(0, qb)
                    attn_block(1, qb)
            elif order == 3:  # qkv b0, then attn b0 interleaved with qkv b1
                for nb in range(4):
                    qkv_block(nb)
                for qb in (3, 2, 1, 0):
                    attn_block(0, qb)
                    if qb == 3:
                        for nb in range(4, 8):
                            qkv_block(nb)
                for qb in (3, 2, 1, 0):
                    attn_block(1, qb)
            elif order == 4:  # dependency-exact: attn(b,qb) right after qkv(nb=qb)
                for b in range(2):
                    for qb in range(4):
                        qkv_block(4 * b + qb)
                        attn_block(b, qb)
            elif order == 6:  # spread qkv(b1) between attn(b0) q-blocks
                for nb in range(4):
                    qkv_block(nb)
                for qb in range(4):
                    attn_block(0, qb)
                    qkv_block(4 + qb)
                for qb in range(4):
                    attn_block(1, qb)
            elif order == 8:  # one-ahead qkv then attn, per batch
                for b in range(2):
                    qkv_block(4 * b)
                    for qb in range(4):
                        if qb < 3:
                            qkv_block(4 * b + qb + 1)
                        attn_block(b, qb)
            elif order == 9:  # one-ahead, b1 qkv spread into attn(b0)
                qkv_block(0)
                for qb in range(4):
                    if qb < 3:
                        qkv_block(qb + 1)
                    attn_block(0, qb)
                    qkv_block(4 + qb)
                for qb in range(4):
                    attn_block(1, qb)
            elif order == 12:  # pre-stage all xt DMAs, alternate qkv/attn
                xts = [xt_load(nb) for nb in range(8)]
                for qb in range(4):
                    qkv_block(qb, xts[qb])
                    attn_block(0, qb)
                    qkv_block(4 + qb, xts[4 + qb])
                for qb in range(4):
                    attn_block(1, qb)
            elif order == 13:  # pre-stage xt, one-ahead like order 9
                xts = [xt_load(nb) for nb in range(8)]
                qkv_block(0, xts[0])
                for qb in range(4):
                    if qb < 3:
                        qkv_block(qb + 1, xts[qb + 1])
                    attn_block(0, qb)
                    qkv_block(4 + qb, xts[4 + qb])
                for qb in range(4):
                    attn_block(1, qb)
            elif order == 10:  # like 9 but b1 attention longest-first
                qkv_block(0)
                for qb in range(4):
                    if qb < 3:
                        qkv_block(qb + 1)
                    attn_block(0, qb)
                    qkv_block(4 + qb)
                for qb in (3, 2, 1, 0):
                    attn_block(1, qb)
            elif order == 11:  # like 9 but b0 attention longest-first too
                qkv_block(0)
                qkv_block(1)
                qkv_block(2)
                qkv_block(3)
                for i, qb in enumerate((3, 2, 1, 0)):
                    attn_block(0, qb)
                    qkv_block(4 + i)
                for qb in (3, 2, 1, 0):
                    attn_block(1, qb)
            elif order == 7:  # qkv(b1) BEFORE each attn(b0) q-block
                for nb in range(4):
                    qkv_block(nb)
                for qb in range(4):
                    qkv_block(4 + qb)
                    attn_block(0, qb)
                for qb in range(4):
                    attn_block(1, qb)

    _split_excess_waits(nc, cap=1)
    return nc


def _get_nc():
    global _NC
    if _NC is None:
        _NC = _build_bass()
    return _NC


def _host_inputs(x, w_qkv, w_proj):
    import ml_dtypes

    bf = ml_dtypes.bfloat16
    x = np.asarray(x, dtype=np.float32)
    w_qkv = np.asarray(w_qkv, dtype=np.float32)
    w_proj = np.asarray(w_proj, dtype=np.float32)

    xT = np.ascontiguousarray(x.reshape(N, D_MODEL).T).astype(bf)

    inv = 1.0 / (ROPE_BASE ** (np.arange(0, HEAD_DIM, 2, dtype=np.float64) / HEAD_DIM))
    ang = np.arange(S, dtype=np.float64)[None, :] * inv[:, None]  # [32, S]
    cosT = np.tile(np.cos(ang), (4, 1)).astype(np.float32)  # [128, S]
    sinT = np.tile(np.sin(ang), (4, 1)).astype(np.float32)

    tri = np.triu(np.ones((128, 128), dtype=np.float32))

    # rot(q)_d = -q_{d+32} (d<32), +q_{d-32} (d>=32), per 64-dim head
    R = np.zeros((64, 64), dtype=np.float32)
    for d in range(32):
        R[d, d + 32] = -1.0
        R[d + 32, d] = 1.0
    Rblk = np.zeros((128, 128), dtype=np.float32)
    Rblk[0:64, 0:64] = R
    Rblk[64:128, 64:128] = R
    rmatT = Rblk.T  # lhsT so out = Rblk @ q
    ident = np.eye(128, dtype=np.float32)

    def rearr_w(w):  # [1024, 128] -> [128, 8*128] rows (c p) -> p, cols (c m)
        return w.reshape(8, 128, 128).transpose(1, 0, 2).reshape(128, 1024)

    in_maps = []
    for c in range(NCORES):
        h0 = 2 * c
        d0 = h0 * HEAD_DIM
        blob = np.zeros((128, CW), dtype=np.float32)
        blob[:, OQ : OQ + 1024] = rearr_w(w_qkv[:, d0 : d0 + 128])
        blob[:, OK : OK + 1024] = rearr_w(w_qkv[:, D_MODEL + d0 : D_MODEL + d0 + 128])
        blob[:, OV : OV + 1024] = rearr_w(
            w_qkv[:, 2 * D_MODEL + d0 : 2 * D_MODEL + d0 + 128]
        )
        blob[:, OP : OP + 1024] = w_proj[d0 : d0 + 128, :]
        blob[:, OC : OC + 2048] = cosT
        blob[:, OS : OS + 2048] = sinT
        blob[:, OT : OT + 128] = tri
        blob[:, OR : OR + 128] = rmatT
        blob[:, OI : OI + 128] = ident
        in_maps.append(dict(xT=xT, consts=blob.astype(bf)))
    return in_maps


def kernel(x, w_qkv, w_proj, b_proj, causal):
    global LAST_RESULT
    assert int(causal) == 1, "kernel is specialized for causal=1"
    _import_concourse()
    from concourse.bass_utils import run_bass_kernel_spmd

    nc = _get_nc()
    in_maps = _host_inputs(x, w_qkv, w_proj)

    kw = {}
    if KERNEL_TRACE:
        kw["trace"] = True
    res = run_bass_kernel_spmd(nc, in_maps, list(range(NCORES)), **kw)
    LAST_RESULT = res

    y = np.zeros((N, D_MODEL), dtype=np.float32)
    for r in res.results:
        y += r["y"].astype(np.float32)
    y += np.asarray(b_proj, dtype=np.float32)[None, :]
    return y.reshape(B, S, D_MODEL)

